# revision 73
# baseline (speedup 1.0000x reference)
"""ApsPool3d TRN2 kernel v11 (final: ~213-218us HW, vs v10 baseline 262us).

Per core (1 batch): input (64, 48, 48, 48) f32 -> output (64, 24, 24, 24) f32.

Architecture (vs v10 baseline):
  - inputs cast f32->bf16 in the DMA (SWDGE), all tiles
  - DVE: 2 y-blur adds only (t1 = d + d>>48row; u = t1 + t1>>48row),
    u written into a guard-column layout (row pitch 50, zeroed guards)
  - PE: full x-blur + z-blur via 3 shifted pumps of u per output chunk
    (W @ u[j-1] + 2W @ u[j] + W @ u[j+1]); W is the block-diag z-blur
    with z-parity-permuted outputs, K-padded to 128; guard columns give
    zero-padding in x for free (no edge repairs)
  - evac PSUM->stored bf16 with (dy,dx)-parity-separated block layout
    [pb=2*dx+dy][y'][x'] per tile (ACT/DVE alternating by knob)
  - squares: batched Act Square+accum (or DVE ttr) per SQG-tile group,
    per parity block, reading contiguous stored blocks
  - argmax phase as baseline (P-matmul + reduce + DRAM bounce + max_index)
  - extraction: 8 predicated HWDGE DMAs (cond = phase==k), static APs,
    straight from stored -> out DRAM (bf16); host casts to f32
"""

import os
import sys

for _p in ("/opt/trn_rl_repo", "/root/.axon_site/_ro/trn_rl_repo"):
    if _p not in sys.path:
        sys.path.insert(0, _p)

import numpy as np

import concourse.bass as bass
import concourse.mybir as mybir
import concourse.tile as tile


# ---- inlined tile_patch (from v10) ----
def _patched_drain_and_barrier(self, tick_clock, wait_clock):
    nc = self.nc
    carrier = mybir.InstNoOp(
        name="tile_drain_wait_carrier",
        engine=mybir.EngineType.SP,
        ins=[],
        outs=[],
    )
    wait_clock.add_sem_waits(
        carrier, tile.ScopedClock({None: tick_clock.global_clock})
    )
    waits = list(carrier.sync_info.on_wait) if carrier.sync_info else []
    for w in waits:
        nop = nc.sync.nop()
        nsi = nop.ins.sync_info
        if nsi is None:
            nop.ins.sync_info = mybir.SyncInfo(on_wait=[w], on_update=[])
        else:
            nsi.on_wait.append(w)
    nc.sync.drain()
    nc.all_engine_barrier()
    assert self.sems is not None
    popped = nc._tile_sem_poison_stack.pop()
    assert popped is self._sem_poison
    nc.clear_and_free_semaphores(list(self.sems.allocated().values()))
    nc.all_engine_barrier()


tile.TileContext._drain_and_barrier = _patched_drain_and_barrier

_SPLIT_SEQ = [0]


def _split_waits(nc, max_waits=1):
    for f in nc.m.functions:
        for bb in f.blocks:
            new_insts = []
            for inst in bb.instructions:
                si = inst.sync_info
                if si is not None and si.on_wait and len(si.on_wait) > max_waits:
                    waits = list(si.on_wait)
                    keep = waits[:max_waits]
                    extras = waits[max_waits:]
                    del si.on_wait[:]
                    si.on_wait.extend(keep)
                    for w in extras:
                        _SPLIT_SEQ[0] += 1
                        nop = mybir.InstNoOp(
                            name=f"waitsplit-{_SPLIT_SEQ[0]}",
                            engine=inst.engine,
                            ins=[],
                            outs=[],
                            sync_info=mybir.SyncInfo(on_wait=[w], on_update=[]),
                        )
                        new_insts.append(nop)
                new_insts.append(inst)
            if len(new_insts) != len(bb.instructions):
                del bb.instructions[:]
                bb.instructions.extend(new_insts)
# ---- end inlined tile_patch ----

from concourse.bass_utils import run_bass_kernel_spmd

F32 = mybir.dt.float32
BF16 = mybir.dt.bfloat16
FP8 = mybir.dt.float8e4
U32 = mybir.dt.uint32
ALU = mybir.AluOpType
ACTF = mybir.ActivationFunctionType

C, N = 64, 48
NH = N // 2  # 24
YX = N * N  # 2304
NT = C // 2  # 32 channel-pair tiles
UP = 50  # u row pitch (48 interior + 2 guard)
UB = 2  # u base offset (4B aligned, element -1 of row 0 is a zero guard)
USZ = UB + N * UP + 2  # 2404
HALF = NH * N  # 1152 columns per half-tile
PB_STRIDE = NH * NH  # 576

STAGE = int(os.environ.get("STAGE", "5"))
SQG = int(os.environ.get("SQG", "4"))  # tiles per squares group
TAILSQ = int(os.environ.get("TAILSQ", "1"))  # final tiles with per-tile squares
NDUMMY = int(os.environ.get("NDUMMY", "0"))  # keep-warm matmuls per tile
SQPRIO = int(os.environ.get("SQPRIO", "30"))  # squares deprioritization offset
EVPRIO = int(os.environ.get("EVPRIO", "10"))  # evac priority boost offset
DMAPRIO = int(os.environ.get("DMAPRIO", "0"))  # input dma priority boost
SQGROUPS = []
_t = 0
while _t < NT - TAILSQ:
    n = min(SQG, NT - TAILSQ - _t)
    SQGROUPS.append((_t, n))
    _t += n
while _t < NT:
    SQGROUPS.append((_t, 1))
    _t += 1
NG = len(SQGROUPS)
_GROUP_END = {t0 + n - 1: (gi, t0, n) for gi, (t0, n) in enumerate(SQGROUPS)}
CAST_MOD = int(os.environ.get("CAST_MOD", "1"))  # t%CAST_MOD==CAST_MOD-1 -> cast dma
# evac engine per (tile,half) index: 'A'=Act 'D'=DVE, cycled
EVAC_PAT = os.environ.get("EVAC_PAT", "DA")
# squares engine per (group,pb) unit: 'A'=Act 'D'=DVE ttr, cycled
SQ_PAT = os.environ.get("SQ_PAT", "A")
# y-add2 engine per tile: 'D'=DVE, 'G'=gpsimd
YADD_PAT = os.environ.get("YADD_PAT", "D")
# if >= 0: unconditional extraction of this phase (debug)
PHASE_FORCE = int(os.environ.get("PHASE_FORCE", "-1"))


def zperm():
    """m (output partition z-slot) -> z_out. Even z' at [0,24), odd at [24,48)."""
    return [2 * i for i in range(NH)] + [2 * i + 1 for i in range(NH)]


def build_weights(filt):
    """W (128,96) bf16 z-blur, K-padded, z-parity-permuted out; P (96,2) f32."""
    f = np.asarray(filt[0, 0], dtype=np.float64)
    s = f.sum()
    kz = f.sum(axis=(1, 2)) / s  # [.25,.5,.25]
    zp = zperm()
    blk = np.zeros((N, N), dtype=np.float64)
    for m in range(N):
        z_out = zp[m]
        for dz in (-1, 0, 1):
            z_in = z_out + dz
            if 0 <= z_in < N:
                blk[z_in, m] = kz[dz + 1] / 16.0  # (1/4 y) * (1/4 x)
    W = np.zeros((128, 96), dtype=np.float64)
    for c in range(2):
        W[c * N : (c + 1) * N, c * N : (c + 1) * N] = blk
    P = np.zeros((96, 2), dtype=np.float32)
    for c in range(2):
        P[c * N : c * N + NH, 0] = 1.0
        P[c * N + NH : c * N + N, 1] = 1.0
    return W.astype(np.float32), P


def build_kernel(nc):
    x = nc.declare_dram_parameter("x", [C, N, YX], F32, isOutput=False)
    w_d = nc.declare_dram_parameter("w", [128, 96], BF16, isOutput=False)
    w2_d = nc.declare_dram_parameter("w2", [128, 96], BF16, isOutput=False)
    par_d = nc.declare_dram_parameter("par", [96, 2], F32, isOutput=False)
    parbf_d = nc.declare_dram_parameter("parbf", [96, 2], BF16, isOutput=False)
    zeros_d = nc.declare_dram_parameter("zeros", [32, USZ], BF16, isOutput=False)
    # out layout: [p=(cl,dz,z') 96, tt*y'*x'] -- winner pb-block for BOTH
    # z-parities; host picks the dz half using out_idx and reorders to
    # (C, NH, NH, NH).
    out = nc.declare_dram_parameter("out", [96, NT * NH * NH], BF16, isOutput=True)
    out_idx = nc.declare_dram_parameter("out_idx", [1, 8], U32, isOutput=True)
    dbg16 = dbgidx = dbg32 = None
    if STAGE in (1, 2, 3):
        dbg16 = nc.declare_dram_parameter("dbg16", [96, 2 * YX], BF16, isOutput=True)
    if STAGE == 4:
        dbg32 = nc.declare_dram_parameter("dbg32", [1, 8], F32, isOutput=True)
        dbgidx = nc.declare_dram_parameter("dbgidx", [1, 8], U32, isOutput=True)

    with tile.TileContext(nc) as tc:
        with (
            tc.tile_pool(name="consts", bufs=1) as consts,
            tc.tile_pool(name="inp", bufs=1) as inp_pool,
            tc.tile_pool(name="work", bufs=1) as work_pool,
            tc.tile_pool(name="ps", bufs=1, space="PSUM") as psum_pool,
            tc.tile_pool(name="store", bufs=1) as store_pool,
            tc.tile_pool(name="dramp", bufs=1, space="DRAM") as dram_pool,
        ):
            w = consts.tile([128, 96], BF16, tag="w")
            w2 = consts.tile([128, 96], BF16, tag="w2")
            par = consts.tile([96, 2], F32, tag="par")
            par_bf = consts.tile([96, 2], BF16, tag="par_bf")
            nc.default_dma_engine.dma_start(w[:], w_d[:])
            nc.default_dma_engine.dma_start(w2[:], w2_d[:])
            nc.default_dma_engine.dma_start(par[:], par_d[:])
            nc.default_dma_engine.dma_start(par_bf[:], parbf_d[:])

            stored = store_pool.tile([96, NT * YX], BF16, tag="stored")
            norm_acc = consts.tile([128, NG * 4], F32, tag="nacc")

            ins32 = [
                inp_pool.tile([96, YX], F32, tag=f"i32_{i}", name=f"i32_{i}")
                for i in range(2)
            ]
            t1s = [
                work_pool.tile([96, YX + N], BF16, tag=f"t1_{i}", name=f"t1_{i}")
                for i in range(2)
            ]
            us = [
                work_pool.tile([128, USZ], BF16, tag=f"u_{i}", name=f"u_{i}")
                for i in range(3)
            ]
            junk = work_pool.tile([96, SQG * PB_STRIDE], FP8, tag="junk", name="junk")

            # zero u tiles once: guards stay zero; partitions 96..127 stay zero
            # (partitions 96..127 zeroed via DMA from host zeros; guards via
            # small strided memsets -- keeps DVE free at startup)
            for uu in us:
                nc.default_dma_engine.dma_start(uu[96:128, :], zeros_d[:])
                nc.vector.memset(uu[0:96, 0:UB], 0.0)
                nc.vector.memset(
                    uu[0:96, UB : UB + N * UP].rearrange("p (y w) -> p y w", w=UP)[
                        :, :, N:UP
                    ],
                    0.0,
                )
                nc.vector.memset(uu[0:96, UB + N * UP : USZ], 0.0)

            # psum half layout: row pitch 64 (cols 48..63 unused) so every
            # 8-row chunk is exactly one 2KB bank (matmul out must not cross
            # a psum bank boundary)
            psums = [
                psum_pool.tile([128, 24 * 64], F32, tag=f"ps_{i}", name=f"ps_{i}")
                for i in range(2)
            ]
            # scratch bank for HAM keep-warm dummy matmuls
            warm_ps = psum_pool.tile([128, 512], F32, tag="warm", name="warm")

            ei = [0]  # evac unit counter
            si = [0]  # squares unit counter
            CHUNKS = ((0, 8), (8, 8), (16, 8))  # (row0, nrows) within half

            for t in range(NT):
                cast = (t % CAST_MOD) == CAST_MOD - 1
                src = x[2 * t : 2 * t + 2].rearrange("c z f -> (c z) f")
                it = ins32[t % 2]
                if cast:
                    d = it[:].bitcast(BF16)[:, 0:YX]
                    with tc.high_priority(offset=DMAPRIO):
                        nc.gpsimd.dma_start(d, src)
                else:
                    nc.default_dma_engine.dma_start(it[:], src)
                    d = it[:]
                t1 = t1s[t % 2]
                u = us[t % 3]

                # ---- y blur ----
                # t1[r] = d[r-1] + d[r], r in [1,48); edges t1[0]=d[0], t1[48]=d[47]
                t1e = t1[:].rearrange("p (b f) -> p b f", f=N)[:, 0 : 49 : 48, :]
                de = d.rearrange("p (b f) -> p b f", f=N)[:, 0 : 48 : 47, :]
                nc.gpsimd.tensor_copy(t1e, de)
                nc.vector.tensor_add(t1[:, N:YX], d[:, 0 : YX - N], d[:, N:YX])
                # u[y] = t1[y] + t1[y+1] into pitch-50 interior
                u_int = u[0:96, UB : UB + N * UP].rearrange(
                    "p (y w) -> p y w", w=UP
                )[:, :, 0:N]
                yeng = nc.vector if YADD_PAT[t % len(YADD_PAT)] == "D" else nc.gpsimd
                yeng.tensor_add(u_int, t1[:, 0:YX], t1[:, N : YX + N])

                if STAGE == 1:
                    if t < 2:
                        uflat = u[0:96, UB : UB + N * UP].rearrange(
                            "p (y w) -> p y w", w=UP
                        )[:, :, 0:N]
                        nc.default_dma_engine.dma_start(
                            dbg16[0:96, t * YX : (t + 1) * YX].rearrange(
                                "p (y w) -> p y w", w=N
                            ),
                            uflat,
                        )
                    continue

                # ---- PE: x+z blur, 3 shifted pumps per chunk ----
                # psum half layout: natural (y_local, x), y_local in [0,24)
                for h in range(2):
                    ps = psums[h]

                    def mv(delta, r0, rn):
                        y0 = h * NH + r0
                        base = UB + delta + y0 * UP
                        return u[0:128, base : base + rn * UP].rearrange(
                            "p (y w) -> p y w", w=UP
                        )[:, :, 0:N]

                    # sequential accumulation groups: each chunk fully
                    # start->accum->stop before the next; each chunk's psum
                    # region is exactly one bank (rows at 64-col pitch)
                    for r0, rn in CHUNKS:
                        pout = ps[0:96, r0 * 64 : (r0 + rn) * 64].rearrange(
                            "p (y w) -> p y w", w=64
                        )[:, :, 0:N]
                        nc.tensor.matmul(
                            pout, w[:], mv(-1, r0, rn), start=True, stop=False
                        )
                        nc.tensor.matmul(
                            pout, w[:], mv(+1, r0, rn), start=False, stop=False
                        )
                        nc.tensor.matmul(
                            pout, w2[:], mv(0, r0, rn), start=False, stop=True
                        )

                    # ---- evac: parity-separating copy, one op per dx ----
                    # psum addr = (2*y2+dy)*64 + 2*x2 + dx
                    # stored: [pb=2dx+dy @ NT*576][tt @576][y'=12h+y2 @24][x2 @1]
                    psv = ps[0:96, 0 : 24 * 64].rearrange(
                        "p (y2 dy2 x2 dx2) -> p dx2 dy2 y2 x2",
                        y2=12, dy2=2, x2=32, dx2=2,
                    )[:, :, :, :, 0:24]
                    stv = stored[0:96, :].rearrange(
                        "p (dx2 dy2 tt y x2) -> p dx2 dy2 y x2 tt",
                        dx2=2, dy2=2, tt=NT, y=NH,
                    )[:, :, :, :, :, t]
                    if STAGE == 2:
                        # plain flat evac for debug: stored = natural (y,x)
                        nc.scalar.copy(
                            stored[
                                0:96, t * YX + h * HALF : t * YX + (h + 1) * HALF
                            ].rearrange("p (y x) -> p y x", x=N),
                            ps[0:96, 0 : 24 * 64].rearrange(
                                "p (y w) -> p y w", w=64
                            )[:, :, 0:N],
                        )
                    else:
                        for dx in range(2):
                            src_v = psv[:, dx]
                            dst_v = stv[:, dx, :, h * 12 : (h + 1) * 12, :]
                            ev = EVAC_PAT[ei[0] % len(EVAC_PAT)]
                            ei[0] += 1
                            # evacs gate PSUM release for the next tile's
                            # matmuls: boost so they schedule ahead of other
                            # queued work on their engine
                            with tc.high_priority(offset=EVPRIO):
                                if ev == "A":
                                    nc.scalar.copy(dst_v, src_v)
                                else:
                                    nc.vector.tensor_copy(dst_v, src_v)
                        # HAM keep-warm: tiny matmul dependent on this half's
                        # evac so it lands in the PE's inter-tile gap
                        if h < NDUMMY:
                            wc = (2 * t + h) % 500
                            nc.tensor.matmul(
                                warm_ps[0:2, wc : wc + 1],
                                par_bf[:, 0:2],
                                stored[0:96, t * PB_STRIDE + h * 288 :][:, 0:1],
                                start=True,
                                stop=True,
                            )

                if STAGE == 2:
                    if t < 2:
                        nc.default_dma_engine.dma_start(
                            dbg16[0:96, t * YX : (t + 1) * YX],
                            stored[0:96, t * YX : (t + 1) * YX],
                        )
                if STAGE == 3:
                    if t < 2:
                        nc.default_dma_engine.dma_start(
                            dbg16[0:96, t * YX : (t + 1) * YX].rearrange(
                                "p (pb f) -> p pb f", pb=4
                            ),
                            stored[0:96, :].rearrange(
                                "p (pb tt f) -> p tt pb f",
                                pb=4, tt=NT, f=PB_STRIDE,
                            )[:, t],
                        )

                # ---- squares per completed group (variable group sizes) ----
                # deprioritized (negative offset = appears later) so pending
                # evacs schedule ahead of the long SQUARE ops on ACT
                if t in _GROUP_END and STAGE != 2:
                    g, t0g, ng = _GROUP_END[t]
                    with tc.high_priority(offset=-SQPRIO):
                        for pb in range(4):
                            acc = norm_acc[0:96, g * 4 + pb : g * 4 + pb + 1]
                            base = pb * NT * PB_STRIDE + t0g * PB_STRIDE
                            gsl = stored[0:96, base : base + ng * PB_STRIDE]
                            nc.scalar.activation(
                                junk[0:96, 0 : ng * PB_STRIDE],
                                gsl,
                                ACTF.Square,
                                accum_out=acc,
                            )

            if STAGE <= 3:
                return

            # ---- finalize norms ----
            zred = psum_pool.tile([2, NG * 4], F32, tag="zred")
            nc.tensor.matmul(
                zred[:], par[:, 0:2], norm_acc[0:96, :], start=True, stop=True
            )
            zred_s = consts.tile([2, NG * 4], F32, tag="zreds")
            nc.scalar.copy(zred_s[:], zred[:])
            nbounce = dram_pool.tile([2, 4], F32, tag="nbounce", name="nbounce")
            zv = zred_s[:].rearrange("p (g c) -> p c g", g=NG)
            n8_2 = consts.tile([2, 4], F32, tag="n8_2")
            nc.vector.tensor_reduce(n8_2[:], zv, mybir.AxisListType.X, ALU.add)
            nc.default_dma_engine.dma_start(nbounce[:], n8_2[:])
            norms8 = consts.tile([1, 8], F32, tag="norms8")
            nc.default_dma_engine.dma_start(
                norms8[:],
                nbounce[:].rearrange("z c -> (z c)").rearrange("(o f) -> o f", o=1),
            )
            nmax = consts.tile([1, 8], F32, tag="nmax")
            nidx = consts.tile([1, 8], U32, tag="nidx")
            nc.vector.max(nmax[:], norms8[:])
            nc.vector.max_index(nidx[:], nmax[:], norms8[:])

            if STAGE == 4:
                nc.default_dma_engine.dma_start(dbg32[:], norms8[:])
                nc.default_dma_engine.dma_start(dbgidx[:], nidx[:])
                return

            # ---- extraction: 8 predicated DMAs stored -> out ----
            nc.default_dma_engine.dma_start(out_idx[:], nidx[:])
            PBSZ = NT * PB_STRIDE  # 18432
            rp = nc.alloc_registers("rp")
            rpb = nc.alloc_registers("rpb")
            nc.regs_load(rp, nidx[0:1, 0:1])
            nc.regs_alu(rpb, rp, 3, ALU.bitwise_and)  # pb = rp & 3
            nc.regs_alu(rpb, rpb, PBSZ, ALU.mult)  # pb * PBSZ
            pb_off = nc.snap(rpb, min_val=0, max_val=3 * PBSZ)
            # gather winner pb block in 4 chunks via dynamic-offset engine
            # copies (DVE/ACT alternating), then plain static DMAs out
            GCH = PBSZ // 8  # 2304
            stgs = [
                work_pool.tile([96, GCH], BF16, tag=f"stg{i}", name=f"stg{i}")
                for i in range(4)
            ]
            for g8 in range(8):
                stg = stgs[g8 % 4]
                if PHASE_FORCE >= 0:
                    src = stored[
                        0:96, (PHASE_FORCE & 3) * PBSZ + g8 * GCH :
                    ][:, 0:GCH]
                else:
                    src = stored[0:96, bass.ds(pb_off, PBSZ)][
                        :, g8 * GCH : (g8 + 1) * GCH
                    ]
                nc.vector.tensor_copy(stg[:], src)
                eng = nc.sync if g8 % 2 == 0 else nc.scalar
                eng.dma_start(out[:, g8 * GCH : (g8 + 1) * GCH], stg[:])


_NC_CACHE = {}


def _get_nc():
    if "nc" not in _NC_CACHE:
        nc = bass.Bass()
        build_kernel(nc)
        _split_waits(nc)
        _NC_CACHE["nc"] = nc
    return _NC_CACHE["nc"]


def run(input_to_pool, filt, trace=False):
    import ml_dtypes

    W, P = build_weights(np.asarray(filt))
    nc = _get_nc()
    x = np.ascontiguousarray(np.asarray(input_to_pool, dtype=np.float32))
    B = x.shape[0]
    in_maps = []
    for b in range(B):
        in_maps.append(
            {
                "x": x[b].reshape(C, N, YX),
                "w": W.astype(ml_dtypes.bfloat16),
                "w2": (2.0 * W).astype(ml_dtypes.bfloat16),
                "par": P,
                "parbf": P.astype(ml_dtypes.bfloat16),
                "zeros": np.zeros((32, USZ), dtype=ml_dtypes.bfloat16),
            }
        )
    res = run_bass_kernel_spmd(nc, in_maps, core_ids=list(range(B)), trace=trace)
    if STAGE >= 5:
        outs = np.empty((B, C, NH, NH, NH), dtype=np.float32)
        for b in range(B):
            o2 = np.asarray(res.results[b]["out"], dtype=np.float32).reshape(
                2, 2, NH, NT, NH, NH
            )  # [cl, dz, z', tt, y', x']
            idx = int(np.asarray(res.results[b]["out_idx"]).reshape(8)[0])
            dz = (idx >> 2) & 1
            # -> out[2*tt+cl, z', y', x']
            outs[b, 0::2] = o2[0, dz].transpose(1, 0, 2, 3)
            outs[b, 1::2] = o2[1, dz].transpose(1, 0, 2, 3)
    else:
        outs = None
    return outs, res


def kernel(input_to_pool, filt, permute_indices=None):
    """Full-input entry point: (8,64,48,48,48) f32 -> (8,64,24,24,24) f32."""
    outs, _ = run(input_to_pool, filt, trace=False)
    return outs


# revision 74
# speedup vs baseline: 1.1339x; 1.1339x over previous
"""ApsPool3d TRN2 kernel v11 (final: ~213-218us HW, vs v10 baseline 262us).

Per core (1 batch): input (64, 48, 48, 48) f32 -> output (64, 24, 24, 24) f32.

Architecture (vs v10 baseline):
  - inputs cast f32->bf16 in the DMA (SWDGE), all tiles
  - DVE: 2 y-blur adds only (t1 = d + d>>48row; u = t1 + t1>>48row),
    u written into a guard-column layout (row pitch 50, zeroed guards)
  - PE: full x-blur + z-blur via 3 shifted pumps of u per output chunk
    (W @ u[j-1] + 2W @ u[j] + W @ u[j+1]); W is the block-diag z-blur
    with z-parity-permuted outputs, K-padded to 128; guard columns give
    zero-padding in x for free (no edge repairs)
  - evac PSUM->stored bf16 with (dy,dx)-parity-separated block layout
    [pb=2*dx+dy][y'][x'] per tile (ACT/DVE alternating by knob)
  - squares: batched Act Square+accum (or DVE ttr) per SQG-tile group,
    per parity block, reading contiguous stored blocks
  - argmax phase as baseline (P-matmul + reduce + DRAM bounce + max_index)
  - extraction: 8 predicated HWDGE DMAs (cond = phase==k), static APs,
    straight from stored -> out DRAM (bf16); host casts to f32
"""

import os
import sys

for _p in ("/opt/trn_rl_repo", "/root/.axon_site/_ro/trn_rl_repo"):
    if _p not in sys.path:
        sys.path.insert(0, _p)

import numpy as np

import concourse.bass as bass
import concourse.mybir as mybir
import concourse.tile as tile


# ---- inlined tile_patch (from v10) ----
def _patched_drain_and_barrier(self, tick_clock, wait_clock):
    nc = self.nc
    carrier = mybir.InstNoOp(
        name="tile_drain_wait_carrier",
        engine=mybir.EngineType.SP,
        ins=[],
        outs=[],
    )
    wait_clock.add_sem_waits(
        carrier, tile.ScopedClock({None: tick_clock.global_clock})
    )
    waits = list(carrier.sync_info.on_wait) if carrier.sync_info else []
    for w in waits:
        nop = nc.sync.nop()
        nsi = nop.ins.sync_info
        if nsi is None:
            nop.ins.sync_info = mybir.SyncInfo(on_wait=[w], on_update=[])
        else:
            nsi.on_wait.append(w)
    nc.sync.drain()
    nc.all_engine_barrier()
    assert self.sems is not None
    popped = nc._tile_sem_poison_stack.pop()
    assert popped is self._sem_poison
    nc.clear_and_free_semaphores(list(self.sems.allocated().values()))
    nc.all_engine_barrier()


tile.TileContext._drain_and_barrier = _patched_drain_and_barrier

_SPLIT_SEQ = [0]


def _split_waits(nc, max_waits=1):
    for f in nc.m.functions:
        for bb in f.blocks:
            new_insts = []
            for inst in bb.instructions:
                si = inst.sync_info
                if si is not None and si.on_wait and len(si.on_wait) > max_waits:
                    waits = list(si.on_wait)
                    keep = waits[:max_waits]
                    extras = waits[max_waits:]
                    del si.on_wait[:]
                    si.on_wait.extend(keep)
                    for w in extras:
                        _SPLIT_SEQ[0] += 1
                        nop = mybir.InstNoOp(
                            name=f"waitsplit-{_SPLIT_SEQ[0]}",
                            engine=inst.engine,
                            ins=[],
                            outs=[],
                            sync_info=mybir.SyncInfo(on_wait=[w], on_update=[]),
                        )
                        new_insts.append(nop)
                new_insts.append(inst)
            if len(new_insts) != len(bb.instructions):
                del bb.instructions[:]
                bb.instructions.extend(new_insts)
# ---- end inlined tile_patch ----

from concourse.bass_utils import run_bass_kernel_spmd

F32 = mybir.dt.float32
BF16 = mybir.dt.bfloat16
FP8 = mybir.dt.float8e4
U32 = mybir.dt.uint32
ALU = mybir.AluOpType
ACTF = mybir.ActivationFunctionType

C, N = 64, 48
NH = N // 2  # 24
YX = N * N  # 2304
NT = C // 2  # 32 channel-pair tiles
UP = 50  # u row pitch (48 interior + 2 guard)
UB = 2  # u base offset (4B aligned, element -1 of row 0 is a zero guard)
USZ = UB + N * UP + 2  # 2404
HALF = NH * N  # 1152 columns per half-tile
PB_STRIDE = NH * NH  # 576

STAGE = int(os.environ.get("STAGE", "5"))
SQG = int(os.environ.get("SQG", "4"))  # tiles per squares group
TAILSQ = int(os.environ.get("TAILSQ", "1"))  # final tiles with per-tile squares
NDUMMY = int(os.environ.get("NDUMMY", "0"))  # keep-warm matmuls per tile
SQPRIO = int(os.environ.get("SQPRIO", "30"))  # squares deprioritization offset
SQGROUPS = []
_t = 0
while _t < NT - TAILSQ:
    n = min(SQG, NT - TAILSQ - _t)
    SQGROUPS.append((_t, n))
    _t += n
while _t < NT:
    SQGROUPS.append((_t, 1))
    _t += 1
NG = len(SQGROUPS)
_GROUP_END = {t0 + n - 1: (gi, t0, n) for gi, (t0, n) in enumerate(SQGROUPS)}
CAST_MOD = int(os.environ.get("CAST_MOD", "1"))  # t%CAST_MOD==CAST_MOD-1 -> cast dma
# evac engine per (tile,half) index: 'A'=Act 'D'=DVE, cycled
EVAC_PAT = os.environ.get("EVAC_PAT", "DA")
# squares engine per (group,pb) unit: 'A'=Act 'D'=DVE ttr, cycled
SQ_PAT = os.environ.get("SQ_PAT", "A")
# y-add2 engine per tile: 'D'=DVE, 'G'=gpsimd
YADD_PAT = os.environ.get("YADD_PAT", "D")
# if >= 0: unconditional extraction of this phase (debug)
PHASE_FORCE = int(os.environ.get("PHASE_FORCE", "-1"))


def zperm():
    """m (output partition z-slot) -> z_out. Even z' at [0,24), odd at [24,48)."""
    return [2 * i for i in range(NH)] + [2 * i + 1 for i in range(NH)]


def build_weights(filt):
    """W (128,96) bf16 z-blur, K-padded, z-parity-permuted out; P (96,2) f32."""
    f = np.asarray(filt[0, 0], dtype=np.float64)
    s = f.sum()
    kz = f.sum(axis=(1, 2)) / s  # [.25,.5,.25]
    zp = zperm()
    blk = np.zeros((N, N), dtype=np.float64)
    for m in range(N):
        z_out = zp[m]
        for dz in (-1, 0, 1):
            z_in = z_out + dz
            if 0 <= z_in < N:
                blk[z_in, m] = kz[dz + 1] / 16.0  # (1/4 y) * (1/4 x)
    W = np.zeros((128, 96), dtype=np.float64)
    for c in range(2):
        W[c * N : (c + 1) * N, c * N : (c + 1) * N] = blk
    P = np.zeros((96, 2), dtype=np.float32)
    for c in range(2):
        P[c * N : c * N + NH, 0] = 1.0
        P[c * N + NH : c * N + N, 1] = 1.0
    return W.astype(np.float32), P


def build_kernel(nc):
    x = nc.declare_dram_parameter("x", [C, N, YX], F32, isOutput=False)
    w_d = nc.declare_dram_parameter("w", [128, 96], BF16, isOutput=False)
    w2_d = nc.declare_dram_parameter("w2", [128, 96], BF16, isOutput=False)
    par_d = nc.declare_dram_parameter("par", [96, 2], F32, isOutput=False)
    parbf_d = nc.declare_dram_parameter("parbf", [96, 2], BF16, isOutput=False)
    zeros_d = nc.declare_dram_parameter("zeros", [32, USZ], BF16, isOutput=False)
    # out layout: [p=(cl,dz,z') 96, tt*y'*x'] -- winner pb-block for BOTH
    # z-parities; host picks the dz half using out_idx and reorders to
    # (C, NH, NH, NH).
    out = nc.declare_dram_parameter("out", [96, NT * NH * NH], BF16, isOutput=True)
    out_idx = nc.declare_dram_parameter("out_idx", [1, 8], U32, isOutput=True)
    dbg16 = dbgidx = dbg32 = None
    if STAGE in (1, 2, 3):
        dbg16 = nc.declare_dram_parameter("dbg16", [96, 2 * YX], BF16, isOutput=True)
    if STAGE == 4:
        dbg32 = nc.declare_dram_parameter("dbg32", [1, 8], F32, isOutput=True)
        dbgidx = nc.declare_dram_parameter("dbgidx", [1, 8], U32, isOutput=True)

    with tile.TileContext(nc) as tc:
        with (
            tc.tile_pool(name="consts", bufs=1) as consts,
            tc.tile_pool(name="inp", bufs=1) as inp_pool,
            tc.tile_pool(name="work", bufs=1) as work_pool,
            tc.tile_pool(name="ps", bufs=1, space="PSUM") as psum_pool,
            tc.tile_pool(name="store", bufs=1) as store_pool,
            tc.tile_pool(name="dramp", bufs=1, space="DRAM") as dram_pool,
        ):
            w = consts.tile([128, 96], BF16, tag="w")
            w2 = consts.tile([128, 96], BF16, tag="w2")
            par = consts.tile([96, 2], F32, tag="par")
            par_bf = consts.tile([96, 2], BF16, tag="par_bf")
            nc.default_dma_engine.dma_start(w[:], w_d[:])
            nc.default_dma_engine.dma_start(w2[:], w2_d[:])
            nc.default_dma_engine.dma_start(par[:], par_d[:])
            nc.default_dma_engine.dma_start(par_bf[:], parbf_d[:])

            stored = store_pool.tile([96, NT * YX], BF16, tag="stored")
            norm_acc = consts.tile([128, NG * 4], F32, tag="nacc")

            ins32 = [
                inp_pool.tile([96, YX], F32, tag=f"i32_{i}", name=f"i32_{i}")
                for i in range(2)
            ]
            t1s = [
                work_pool.tile([96, YX + N], BF16, tag=f"t1_{i}", name=f"t1_{i}")
                for i in range(2)
            ]
            us = [
                work_pool.tile([128, USZ], BF16, tag=f"u_{i}", name=f"u_{i}")
                for i in range(3)
            ]
            junk = work_pool.tile([96, SQG * PB_STRIDE], FP8, tag="junk", name="junk")

            # zero u tiles once: guards stay zero; partitions 96..127 stay zero
            # (partitions 96..127 zeroed via DMA from host zeros; guards via
            # small strided memsets -- keeps DVE free at startup)
            for uu in us:
                nc.default_dma_engine.dma_start(uu[96:128, :], zeros_d[:])
                nc.vector.memset(uu[0:96, 0:UB], 0.0)
                nc.vector.memset(
                    uu[0:96, UB : UB + N * UP].rearrange("p (y w) -> p y w", w=UP)[
                        :, :, N:UP
                    ],
                    0.0,
                )
                nc.vector.memset(uu[0:96, UB + N * UP : USZ], 0.0)

            # psum half layout: row pitch 64 (cols 48..63 unused) so every
            # 8-row chunk is exactly one 2KB bank (matmul out must not cross
            # a psum bank boundary)
            psums = [
                psum_pool.tile([128, 24 * 64], F32, tag=f"ps_{i}", name=f"ps_{i}")
                for i in range(2)
            ]
            # scratch bank for HAM keep-warm dummy matmuls
            warm_ps = psum_pool.tile([128, 512], F32, tag="warm", name="warm")

            ei = [0]  # evac unit counter
            si = [0]  # squares unit counter
            CHUNKS = ((0, 8), (8, 8), (16, 8))  # (row0, nrows) within half

            for t in range(NT):
                cast = (t % CAST_MOD) == CAST_MOD - 1
                src = x[2 * t : 2 * t + 2].rearrange("c z f -> (c z) f")
                it = ins32[t % 2]
                if cast:
                    d = it[:].bitcast(BF16)[:, 0:YX]
                    nc.gpsimd.dma_start(d, src)
                else:
                    nc.default_dma_engine.dma_start(it[:], src)
                    d = it[:]
                t1 = t1s[t % 2]
                u = us[t % 3]

                # ---- y blur ----
                # t1[r] = d[r-1] + d[r], r in [1,48); edges t1[0]=d[0], t1[48]=d[47]
                t1e = t1[:].rearrange("p (b f) -> p b f", f=N)[:, 0 : 49 : 48, :]
                de = d.rearrange("p (b f) -> p b f", f=N)[:, 0 : 48 : 47, :]
                nc.gpsimd.tensor_copy(t1e, de)
                nc.vector.tensor_add(t1[:, N:YX], d[:, 0 : YX - N], d[:, N:YX])
                # u[y] = t1[y] + t1[y+1] into pitch-50 interior
                u_int = u[0:96, UB : UB + N * UP].rearrange(
                    "p (y w) -> p y w", w=UP
                )[:, :, 0:N]
                yeng = nc.vector if YADD_PAT[t % len(YADD_PAT)] == "D" else nc.gpsimd
                yeng.tensor_add(u_int, t1[:, 0:YX], t1[:, N : YX + N])

                if STAGE == 1:
                    if t < 2:
                        uflat = u[0:96, UB : UB + N * UP].rearrange(
                            "p (y w) -> p y w", w=UP
                        )[:, :, 0:N]
                        nc.default_dma_engine.dma_start(
                            dbg16[0:96, t * YX : (t + 1) * YX].rearrange(
                                "p (y w) -> p y w", w=N
                            ),
                            uflat,
                        )
                    continue

                # ---- PE: x+z blur, 3 shifted pumps per chunk ----
                # psum half layout: natural (y_local, x), y_local in [0,24)
                for h in range(2):
                    ps = psums[h]

                    def mv(delta, r0, rn):
                        y0 = h * NH + r0
                        base = UB + delta + y0 * UP
                        return u[0:128, base : base + rn * UP].rearrange(
                            "p (y w) -> p y w", w=UP
                        )[:, :, 0:N]

                    # sequential accumulation groups: each chunk fully
                    # start->accum->stop before the next; each chunk's psum
                    # region is exactly one bank (rows at 64-col pitch)
                    for r0, rn in CHUNKS:
                        pout = ps[0:96, r0 * 64 : (r0 + rn) * 64].rearrange(
                            "p (y w) -> p y w", w=64
                        )[:, :, 0:N]
                        nc.tensor.matmul(
                            pout, w[:], mv(-1, r0, rn), start=True, stop=False
                        )
                        nc.tensor.matmul(
                            pout, w[:], mv(+1, r0, rn), start=False, stop=False
                        )
                        nc.tensor.matmul(
                            pout, w2[:], mv(0, r0, rn), start=False, stop=True
                        )

                    # ---- evac: parity-separating copy, one op per dx ----
                    # psum addr = (2*y2+dy)*64 + 2*x2 + dx
                    # stored: [pb=2dx+dy @ NT*576][tt @576][y'=12h+y2 @24][x2 @1]
                    psv = ps[0:96, 0 : 24 * 64].rearrange(
                        "p (y2 dy2 x2 dx2) -> p dx2 dy2 y2 x2",
                        y2=12, dy2=2, x2=32, dx2=2,
                    )[:, :, :, :, 0:24]
                    stv = stored[0:96, :].rearrange(
                        "p (dx2 dy2 tt y x2) -> p dx2 dy2 y x2 tt",
                        dx2=2, dy2=2, tt=NT, y=NH,
                    )[:, :, :, :, :, t]
                    if STAGE == 2:
                        # plain flat evac for debug: stored = natural (y,x)
                        nc.scalar.copy(
                            stored[
                                0:96, t * YX + h * HALF : t * YX + (h + 1) * HALF
                            ].rearrange("p (y x) -> p y x", x=N),
                            ps[0:96, 0 : 24 * 64].rearrange(
                                "p (y w) -> p y w", w=64
                            )[:, :, 0:N],
                        )
                    else:
                        for dx in range(2):
                            src_v = psv[:, dx]
                            dst_v = stv[:, dx, :, h * 12 : (h + 1) * 12, :]
                            ev = EVAC_PAT[ei[0] % len(EVAC_PAT)]
                            ei[0] += 1
                            if ev == "A":
                                nc.scalar.copy(dst_v, src_v)
                            else:
                                nc.vector.tensor_copy(dst_v, src_v)
                        # HAM keep-warm: tiny matmul dependent on this half's
                        # evac so it lands in the PE's inter-tile gap
                        if h < NDUMMY:
                            wc = (2 * t + h) % 500
                            nc.tensor.matmul(
                                warm_ps[0:2, wc : wc + 1],
                                par_bf[:, 0:2],
                                stored[0:96, t * PB_STRIDE + h * 288 :][:, 0:1],
                                start=True,
                                stop=True,
                            )

                if STAGE == 2:
                    if t < 2:
                        nc.default_dma_engine.dma_start(
                            dbg16[0:96, t * YX : (t + 1) * YX],
                            stored[0:96, t * YX : (t + 1) * YX],
                        )
                if STAGE == 3:
                    if t < 2:
                        nc.default_dma_engine.dma_start(
                            dbg16[0:96, t * YX : (t + 1) * YX].rearrange(
                                "p (pb f) -> p pb f", pb=4
                            ),
                            stored[0:96, :].rearrange(
                                "p (pb tt f) -> p tt pb f",
                                pb=4, tt=NT, f=PB_STRIDE,
                            )[:, t],
                        )

                # ---- squares per completed group (variable group sizes) ----
                # deprioritized (negative offset = appears later) so pending
                # evacs schedule ahead of the long SQUARE ops on ACT
                if t in _GROUP_END and STAGE != 2:
                    g, t0g, ng = _GROUP_END[t]
                    with tc.high_priority(offset=-SQPRIO):
                        for pb in range(4):
                            acc = norm_acc[0:96, g * 4 + pb : g * 4 + pb + 1]
                            base = pb * NT * PB_STRIDE + t0g * PB_STRIDE
                            gsl = stored[0:96, base : base + ng * PB_STRIDE]
                            nc.scalar.activation(
                                junk[0:96, 0 : ng * PB_STRIDE],
                                gsl,
                                ACTF.Square,
                                accum_out=acc,
                            )

            if STAGE <= 3:
                return

            # ---- finalize norms ----
            zred = psum_pool.tile([2, NG * 4], F32, tag="zred")
            nc.tensor.matmul(
                zred[:], par[:, 0:2], norm_acc[0:96, :], start=True, stop=True
            )
            zred_s = consts.tile([2, NG * 4], F32, tag="zreds")
            nc.scalar.copy(zred_s[:], zred[:])
            nbounce = dram_pool.tile([2, 4], F32, tag="nbounce", name="nbounce")
            zv = zred_s[:].rearrange("p (g c) -> p c g", g=NG)
            n8_2 = consts.tile([2, 4], F32, tag="n8_2")
            nc.vector.tensor_reduce(n8_2[:], zv, mybir.AxisListType.X, ALU.add)
            nc.default_dma_engine.dma_start(nbounce[:], n8_2[:])
            norms8 = consts.tile([1, 8], F32, tag="norms8")
            nc.default_dma_engine.dma_start(
                norms8[:],
                nbounce[:].rearrange("z c -> (z c)").rearrange("(o f) -> o f", o=1),
            )
            nmax = consts.tile([1, 8], F32, tag="nmax")
            nidx = consts.tile([1, 8], U32, tag="nidx")
            nc.vector.max(nmax[:], norms8[:])
            nc.vector.max_index(nidx[:], nmax[:], norms8[:])

            if STAGE == 4:
                nc.default_dma_engine.dma_start(dbg32[:], norms8[:])
                nc.default_dma_engine.dma_start(dbgidx[:], nidx[:])
                return

            # ---- extraction: 8 predicated DMAs stored -> out ----
            nc.default_dma_engine.dma_start(out_idx[:], nidx[:])
            PBSZ = NT * PB_STRIDE  # 18432
            rp = nc.alloc_registers("rp")
            rpb = nc.alloc_registers("rpb")
            nc.regs_load(rp, nidx[0:1, 0:1])
            nc.regs_alu(rpb, rp, 3, ALU.bitwise_and)  # pb = rp & 3
            nc.regs_alu(rpb, rpb, PBSZ, ALU.mult)  # pb * PBSZ
            pb_off = nc.snap(rpb, min_val=0, max_val=3 * PBSZ)
            # gather winner pb block in 4 chunks via dynamic-offset engine
            # copies (DVE/ACT alternating), then plain static DMAs out
            GCH = PBSZ // 8  # 2304
            stgs = [
                work_pool.tile([96, GCH], BF16, tag=f"stg{i}", name=f"stg{i}")
                for i in range(4)
            ]
            for g8 in range(8):
                stg = stgs[g8 % 4]
                if PHASE_FORCE >= 0:
                    src = stored[
                        0:96, (PHASE_FORCE & 3) * PBSZ + g8 * GCH :
                    ][:, 0:GCH]
                else:
                    src = stored[0:96, bass.ds(pb_off, PBSZ)][
                        :, g8 * GCH : (g8 + 1) * GCH
                    ]
                nc.vector.tensor_copy(stg[:], src)
                eng = nc.sync if g8 % 2 == 0 else nc.scalar
                eng.dma_start(out[:, g8 * GCH : (g8 + 1) * GCH], stg[:])


_NC_CACHE = {}


def _get_nc():
    if "nc" not in _NC_CACHE:
        nc = bass.Bass()
        build_kernel(nc)
        _split_waits(nc)
        _NC_CACHE["nc"] = nc
    return _NC_CACHE["nc"]


def run(input_to_pool, filt, trace=False):
    import ml_dtypes

    W, P = build_weights(np.asarray(filt))
    nc = _get_nc()
    x = np.ascontiguousarray(np.asarray(input_to_pool, dtype=np.float32))
    B = x.shape[0]
    in_maps = []
    for b in range(B):
        in_maps.append(
            {
                "x": x[b].reshape(C, N, YX),
                "w": W.astype(ml_dtypes.bfloat16),
                "w2": (2.0 * W).astype(ml_dtypes.bfloat16),
                "par": P,
                "parbf": P.astype(ml_dtypes.bfloat16),
                "zeros": np.zeros((32, USZ), dtype=ml_dtypes.bfloat16),
            }
        )
    res = run_bass_kernel_spmd(nc, in_maps, core_ids=list(range(B)), trace=trace)
    if STAGE >= 5:
        outs = np.empty((B, C, NH, NH, NH), dtype=np.float32)
        for b in range(B):
            o2 = np.asarray(res.results[b]["out"], dtype=np.float32).reshape(
                2, 2, NH, NT, NH, NH
            )  # [cl, dz, z', tt, y', x']
            idx = int(np.asarray(res.results[b]["out_idx"]).reshape(8)[0])
            dz = (idx >> 2) & 1
            # -> out[2*tt+cl, z', y', x']
            outs[b, 0::2] = o2[0, dz].transpose(1, 0, 2, 3)
            outs[b, 1::2] = o2[1, dz].transpose(1, 0, 2, 3)
    else:
        outs = None
    return outs, res


def kernel(input_to_pool, filt, permute_indices=None):
    """Full-input entry point: (8,64,48,48,48) f32 -> (8,64,24,24,24) f32."""
    outs, _ = run(input_to_pool, filt, trace=False)
    return outs


# revision 76
# speedup vs baseline: 1.1372x; 1.0029x over previous
"""ApsPool3d TRN2 kernel v11 (final: ~213-218us HW, vs v10 baseline 262us).

Per core (1 batch): input (64, 48, 48, 48) f32 -> output (64, 24, 24, 24) f32.

Architecture (vs v10 baseline):
  - inputs cast f32->bf16 in the DMA (SWDGE), all tiles
  - DVE: 2 y-blur adds only (t1 = d + d>>48row; u = t1 + t1>>48row),
    u written into a guard-column layout (row pitch 50, zeroed guards)
  - PE: full x-blur + z-blur via 3 shifted pumps of u per output chunk
    (W @ u[j-1] + 2W @ u[j] + W @ u[j+1]); W is the block-diag z-blur
    with z-parity-permuted outputs, K-padded to 128; guard columns give
    zero-padding in x for free (no edge repairs)
  - evac PSUM->stored bf16 with (dy,dx)-parity-separated block layout
    [pb=2*dx+dy][y'][x'] per tile (ACT/DVE alternating by knob)
  - squares: batched Act Square+accum (or DVE ttr) per SQG-tile group,
    per parity block, reading contiguous stored blocks
  - argmax phase as baseline (P-matmul + reduce + DRAM bounce + max_index)
  - extraction: 8 predicated HWDGE DMAs (cond = phase==k), static APs,
    straight from stored -> out DRAM (bf16); host casts to f32
"""

import os
import sys

for _p in ("/opt/trn_rl_repo", "/root/.axon_site/_ro/trn_rl_repo"):
    if _p not in sys.path:
        sys.path.insert(0, _p)

import numpy as np

import concourse.bass as bass
import concourse.mybir as mybir
import concourse.tile as tile


# ---- inlined tile_patch (from v10) ----
def _patched_drain_and_barrier(self, tick_clock, wait_clock):
    nc = self.nc
    carrier = mybir.InstNoOp(
        name="tile_drain_wait_carrier",
        engine=mybir.EngineType.SP,
        ins=[],
        outs=[],
    )
    wait_clock.add_sem_waits(
        carrier, tile.ScopedClock({None: tick_clock.global_clock})
    )
    waits = list(carrier.sync_info.on_wait) if carrier.sync_info else []
    for w in waits:
        nop = nc.sync.nop()
        nsi = nop.ins.sync_info
        if nsi is None:
            nop.ins.sync_info = mybir.SyncInfo(on_wait=[w], on_update=[])
        else:
            nsi.on_wait.append(w)
    nc.sync.drain()
    nc.all_engine_barrier()
    assert self.sems is not None
    popped = nc._tile_sem_poison_stack.pop()
    assert popped is self._sem_poison
    nc.clear_and_free_semaphores(list(self.sems.allocated().values()))
    nc.all_engine_barrier()


tile.TileContext._drain_and_barrier = _patched_drain_and_barrier

_SPLIT_SEQ = [0]


def _split_waits(nc, max_waits=1):
    for f in nc.m.functions:
        for bb in f.blocks:
            new_insts = []
            for inst in bb.instructions:
                si = inst.sync_info
                if si is not None and si.on_wait and len(si.on_wait) > max_waits:
                    waits = list(si.on_wait)
                    keep = waits[:max_waits]
                    extras = waits[max_waits:]
                    del si.on_wait[:]
                    si.on_wait.extend(keep)
                    for w in extras:
                        _SPLIT_SEQ[0] += 1
                        nop = mybir.InstNoOp(
                            name=f"waitsplit-{_SPLIT_SEQ[0]}",
                            engine=inst.engine,
                            ins=[],
                            outs=[],
                            sync_info=mybir.SyncInfo(on_wait=[w], on_update=[]),
                        )
                        new_insts.append(nop)
                new_insts.append(inst)
            if len(new_insts) != len(bb.instructions):
                del bb.instructions[:]
                bb.instructions.extend(new_insts)
# ---- end inlined tile_patch ----

from concourse.bass_utils import run_bass_kernel_spmd

F32 = mybir.dt.float32
BF16 = mybir.dt.bfloat16
FP8 = mybir.dt.float8e4
U32 = mybir.dt.uint32
ALU = mybir.AluOpType
ACTF = mybir.ActivationFunctionType

C, N = 64, 48
NH = N // 2  # 24
YX = N * N  # 2304
NT = C // 2  # 32 channel-pair tiles
UP = 50  # u row pitch (48 interior + 2 guard)
UB = 2  # u base offset (4B aligned, element -1 of row 0 is a zero guard)
USZ = UB + N * UP + 2  # 2404
HALF = NH * N  # 1152 columns per half-tile
PB_STRIDE = NH * NH  # 576

STAGE = int(os.environ.get("STAGE", "5"))
SQG = int(os.environ.get("SQG", "4"))  # tiles per squares group
TAILSQ = int(os.environ.get("TAILSQ", "1"))  # final tiles with per-tile squares
NDUMMY = int(os.environ.get("NDUMMY", "0"))  # keep-warm matmuls per tile
SQPRIO = int(os.environ.get("SQPRIO", "30"))  # squares deprioritization offset
SQGROUPS = []
_t = 0
while _t < NT - TAILSQ:
    n = min(SQG, NT - TAILSQ - _t)
    SQGROUPS.append((_t, n))
    _t += n
while _t < NT:
    SQGROUPS.append((_t, 1))
    _t += 1
NG = len(SQGROUPS)
_GROUP_END = {t0 + n - 1: (gi, t0, n) for gi, (t0, n) in enumerate(SQGROUPS)}
CAST_MOD = int(os.environ.get("CAST_MOD", "1"))  # t%CAST_MOD==CAST_MOD-1 -> cast dma
# evac engine per (tile,half) index: 'A'=Act 'D'=DVE, cycled
EVAC_PAT = os.environ.get("EVAC_PAT", "DA")
# squares engine per (group,pb) unit: 'A'=Act 'D'=DVE ttr, cycled
SQ_PAT = os.environ.get("SQ_PAT", "A")
# y-add2 engine per tile: 'D'=DVE, 'G'=gpsimd
YADD_PAT = os.environ.get("YADD_PAT", "D")
# if >= 0: unconditional extraction of this phase (debug)
PHASE_FORCE = int(os.environ.get("PHASE_FORCE", "-1"))


def zperm():
    """m (output partition z-slot) -> z_out. Even z' at [0,24), odd at [24,48)."""
    return [2 * i for i in range(NH)] + [2 * i + 1 for i in range(NH)]


def build_weights(filt):
    """W (128,96) bf16 z-blur, K-padded, z-parity-permuted out; P (96,2) f32."""
    f = np.asarray(filt[0, 0], dtype=np.float64)
    s = f.sum()
    kz = f.sum(axis=(1, 2)) / s  # [.25,.5,.25]
    zp = zperm()
    blk = np.zeros((N, N), dtype=np.float64)
    for m in range(N):
        z_out = zp[m]
        for dz in (-1, 0, 1):
            z_in = z_out + dz
            if 0 <= z_in < N:
                blk[z_in, m] = kz[dz + 1] / 16.0  # (1/4 y) * (1/4 x)
    W = np.zeros((128, 96), dtype=np.float64)
    for c in range(2):
        W[c * N : (c + 1) * N, c * N : (c + 1) * N] = blk
    P = np.zeros((96, 2), dtype=np.float32)
    for c in range(2):
        P[c * N : c * N + NH, 0] = 1.0
        P[c * N + NH : c * N + N, 1] = 1.0
    return W.astype(np.float32), P


def build_kernel(nc):
    x = nc.declare_dram_parameter("x", [C, N, YX], F32, isOutput=False)
    w_d = nc.declare_dram_parameter("w", [128, 96], BF16, isOutput=False)
    w2_d = nc.declare_dram_parameter("w2", [128, 96], BF16, isOutput=False)
    par_d = nc.declare_dram_parameter("par", [96, 2], F32, isOutput=False)
    parbf_d = nc.declare_dram_parameter("parbf", [96, 2], BF16, isOutput=False)
    zeros_d = nc.declare_dram_parameter("zeros", [32, USZ], BF16, isOutput=False)
    # out layout: [p=(cl,dz,z') 96, tt*y'*x'] -- winner pb-block for BOTH
    # z-parities; host picks the dz half using out_idx and reorders to
    # (C, NH, NH, NH).
    out = nc.declare_dram_parameter("out", [96, NT * NH * NH], BF16, isOutput=True)
    out_idx = nc.declare_dram_parameter("out_idx", [1, 8], U32, isOutput=True)
    dbg16 = dbgidx = dbg32 = None
    if STAGE in (1, 2, 3):
        dbg16 = nc.declare_dram_parameter("dbg16", [96, 2 * YX], BF16, isOutput=True)
    if STAGE == 4:
        dbg32 = nc.declare_dram_parameter("dbg32", [1, 8], F32, isOutput=True)
        dbgidx = nc.declare_dram_parameter("dbgidx", [1, 8], U32, isOutput=True)

    with tile.TileContext(nc) as tc:
        with (
            tc.tile_pool(name="consts", bufs=1) as consts,
            tc.tile_pool(name="inp", bufs=1) as inp_pool,
            tc.tile_pool(name="work", bufs=1) as work_pool,
            tc.tile_pool(name="ps", bufs=1, space="PSUM") as psum_pool,
            tc.tile_pool(name="store", bufs=1) as store_pool,
            tc.tile_pool(name="dramp", bufs=1, space="DRAM") as dram_pool,
        ):
            w = consts.tile([128, 96], BF16, tag="w")
            w2 = consts.tile([128, 96], BF16, tag="w2")
            par = consts.tile([96, 2], F32, tag="par")
            par_bf = consts.tile([96, 2], BF16, tag="par_bf")

            ins32 = [
                inp_pool.tile([96, YX], F32, tag=f"i32_{i}", name=f"i32_{i}")
                for i in range(2)
            ]
            # prefetch the first two input tiles ahead of the const loads so
            # the pipeline-priming DMAs start immediately (shortens the ramp)
            NPRE = 2 if CAST_MOD == 1 else 0
            for t in range(NPRE):
                src = x[2 * t : 2 * t + 2].rearrange("c z f -> (c z) f")
                dpre = ins32[t % 2][:].bitcast(BF16)[:, 0:YX]
                nc.gpsimd.dma_start(dpre, src)

            nc.default_dma_engine.dma_start(w[:], w_d[:])
            nc.default_dma_engine.dma_start(w2[:], w2_d[:])
            nc.default_dma_engine.dma_start(par[:], par_d[:])
            nc.default_dma_engine.dma_start(par_bf[:], parbf_d[:])

            stored = store_pool.tile([96, NT * YX], BF16, tag="stored")
            norm_acc = consts.tile([128, NG * 4], F32, tag="nacc")
            t1s = [
                work_pool.tile([96, YX + N], BF16, tag=f"t1_{i}", name=f"t1_{i}")
                for i in range(2)
            ]
            us = [
                work_pool.tile([128, USZ], BF16, tag=f"u_{i}", name=f"u_{i}")
                for i in range(3)
            ]
            junk = work_pool.tile([96, SQG * PB_STRIDE], FP8, tag="junk", name="junk")

            # zero u tiles once: guards stay zero; partitions 96..127 stay zero
            # (partitions 96..127 zeroed via DMA from host zeros; guards via
            # small strided memsets -- keeps DVE free at startup)
            for uu in us:
                nc.default_dma_engine.dma_start(uu[96:128, :], zeros_d[:])
                nc.vector.memset(uu[0:96, 0:UB], 0.0)
                nc.vector.memset(
                    uu[0:96, UB : UB + N * UP].rearrange("p (y w) -> p y w", w=UP)[
                        :, :, N:UP
                    ],
                    0.0,
                )
                nc.vector.memset(uu[0:96, UB + N * UP : USZ], 0.0)

            # psum half layout: row pitch 64 (cols 48..63 unused) so every
            # 8-row chunk is exactly one 2KB bank (matmul out must not cross
            # a psum bank boundary)
            psums = [
                psum_pool.tile([128, 24 * 64], F32, tag=f"ps_{i}", name=f"ps_{i}")
                for i in range(2)
            ]
            # scratch bank for HAM keep-warm dummy matmuls
            warm_ps = psum_pool.tile([128, 512], F32, tag="warm", name="warm")

            ei = [0]  # evac unit counter
            si = [0]  # squares unit counter
            CHUNKS = ((0, 8), (8, 8), (16, 8))  # (row0, nrows) within half

            for t in range(NT):
                cast = (t % CAST_MOD) == CAST_MOD - 1
                src = x[2 * t : 2 * t + 2].rearrange("c z f -> (c z) f")
                it = ins32[t % 2]
                if cast:
                    d = it[:].bitcast(BF16)[:, 0:YX]
                    if t >= NPRE:
                        nc.gpsimd.dma_start(d, src)
                else:
                    nc.default_dma_engine.dma_start(it[:], src)
                    d = it[:]
                t1 = t1s[t % 2]
                u = us[t % 3]

                # ---- y blur ----
                # t1[r] = d[r-1] + d[r], r in [1,48); edges t1[0]=d[0], t1[48]=d[47]
                t1e = t1[:].rearrange("p (b f) -> p b f", f=N)[:, 0 : 49 : 48, :]
                de = d.rearrange("p (b f) -> p b f", f=N)[:, 0 : 48 : 47, :]
                nc.gpsimd.tensor_copy(t1e, de)
                nc.vector.tensor_add(t1[:, N:YX], d[:, 0 : YX - N], d[:, N:YX])
                # u[y] = t1[y] + t1[y+1] into pitch-50 interior
                u_int = u[0:96, UB : UB + N * UP].rearrange(
                    "p (y w) -> p y w", w=UP
                )[:, :, 0:N]
                yeng = nc.vector if YADD_PAT[t % len(YADD_PAT)] == "D" else nc.gpsimd
                yeng.tensor_add(u_int, t1[:, 0:YX], t1[:, N : YX + N])

                if STAGE == 1:
                    if t < 2:
                        uflat = u[0:96, UB : UB + N * UP].rearrange(
                            "p (y w) -> p y w", w=UP
                        )[:, :, 0:N]
                        nc.default_dma_engine.dma_start(
                            dbg16[0:96, t * YX : (t + 1) * YX].rearrange(
                                "p (y w) -> p y w", w=N
                            ),
                            uflat,
                        )
                    continue

                # ---- PE: x+z blur, 3 shifted pumps per chunk ----
                # psum half layout: natural (y_local, x), y_local in [0,24)
                for h in range(2):
                    ps = psums[h]

                    def mv(delta, r0, rn):
                        y0 = h * NH + r0
                        base = UB + delta + y0 * UP
                        return u[0:128, base : base + rn * UP].rearrange(
                            "p (y w) -> p y w", w=UP
                        )[:, :, 0:N]

                    # sequential accumulation groups: each chunk fully
                    # start->accum->stop before the next; each chunk's psum
                    # region is exactly one bank (rows at 64-col pitch)
                    for r0, rn in CHUNKS:
                        pout = ps[0:96, r0 * 64 : (r0 + rn) * 64].rearrange(
                            "p (y w) -> p y w", w=64
                        )[:, :, 0:N]
                        nc.tensor.matmul(
                            pout, w[:], mv(-1, r0, rn), start=True, stop=False
                        )
                        nc.tensor.matmul(
                            pout, w[:], mv(+1, r0, rn), start=False, stop=False
                        )
                        nc.tensor.matmul(
                            pout, w2[:], mv(0, r0, rn), start=False, stop=True
                        )

                    # ---- evac: parity-separating copy, one op per dx ----
                    # psum addr = (2*y2+dy)*64 + 2*x2 + dx
                    # stored: [pb=2dx+dy @ NT*576][tt @576][y'=12h+y2 @24][x2 @1]
                    psv = ps[0:96, 0 : 24 * 64].rearrange(
                        "p (y2 dy2 x2 dx2) -> p dx2 dy2 y2 x2",
                        y2=12, dy2=2, x2=32, dx2=2,
                    )[:, :, :, :, 0:24]
                    stv = stored[0:96, :].rearrange(
                        "p (dx2 dy2 tt y x2) -> p dx2 dy2 y x2 tt",
                        dx2=2, dy2=2, tt=NT, y=NH,
                    )[:, :, :, :, :, t]
                    if STAGE == 2:
                        # plain flat evac for debug: stored = natural (y,x)
                        nc.scalar.copy(
                            stored[
                                0:96, t * YX + h * HALF : t * YX + (h + 1) * HALF
                            ].rearrange("p (y x) -> p y x", x=N),
                            ps[0:96, 0 : 24 * 64].rearrange(
                                "p (y w) -> p y w", w=64
                            )[:, :, 0:N],
                        )
                    else:
                        for dx in range(2):
                            src_v = psv[:, dx]
                            dst_v = stv[:, dx, :, h * 12 : (h + 1) * 12, :]
                            ev = EVAC_PAT[ei[0] % len(EVAC_PAT)]
                            ei[0] += 1
                            if ev == "A":
                                nc.scalar.copy(dst_v, src_v)
                            else:
                                nc.vector.tensor_copy(dst_v, src_v)
                        # HAM keep-warm: tiny matmul dependent on this half's
                        # evac so it lands in the PE's inter-tile gap
                        if h < NDUMMY:
                            wc = (2 * t + h) % 500
                            nc.tensor.matmul(
                                warm_ps[0:2, wc : wc + 1],
                                par_bf[:, 0:2],
                                stored[0:96, t * PB_STRIDE + h * 288 :][:, 0:1],
                                start=True,
                                stop=True,
                            )

                if STAGE == 2:
                    if t < 2:
                        nc.default_dma_engine.dma_start(
                            dbg16[0:96, t * YX : (t + 1) * YX],
                            stored[0:96, t * YX : (t + 1) * YX],
                        )
                if STAGE == 3:
                    if t < 2:
                        nc.default_dma_engine.dma_start(
                            dbg16[0:96, t * YX : (t + 1) * YX].rearrange(
                                "p (pb f) -> p pb f", pb=4
                            ),
                            stored[0:96, :].rearrange(
                                "p (pb tt f) -> p tt pb f",
                                pb=4, tt=NT, f=PB_STRIDE,
                            )[:, t],
                        )

                # ---- squares per completed group (variable group sizes) ----
                # deprioritized (negative offset = appears later) so pending
                # evacs schedule ahead of the long SQUARE ops on ACT
                if t in _GROUP_END and STAGE != 2:
                    g, t0g, ng = _GROUP_END[t]
                    with tc.high_priority(offset=-SQPRIO):
                        for pb in range(4):
                            acc = norm_acc[0:96, g * 4 + pb : g * 4 + pb + 1]
                            base = pb * NT * PB_STRIDE + t0g * PB_STRIDE
                            gsl = stored[0:96, base : base + ng * PB_STRIDE]
                            nc.scalar.activation(
                                junk[0:96, 0 : ng * PB_STRIDE],
                                gsl,
                                ACTF.Square,
                                accum_out=acc,
                            )

            if STAGE <= 3:
                return

            # ---- finalize norms ----
            zred = psum_pool.tile([2, NG * 4], F32, tag="zred")
            nc.tensor.matmul(
                zred[:], par[:, 0:2], norm_acc[0:96, :], start=True, stop=True
            )
            zred_s = consts.tile([2, NG * 4], F32, tag="zreds")
            nc.scalar.copy(zred_s[:], zred[:])
            nbounce = dram_pool.tile([2, 4], F32, tag="nbounce", name="nbounce")
            zv = zred_s[:].rearrange("p (g c) -> p c g", g=NG)
            n8_2 = consts.tile([2, 4], F32, tag="n8_2")
            nc.vector.tensor_reduce(n8_2[:], zv, mybir.AxisListType.X, ALU.add)
            nc.default_dma_engine.dma_start(nbounce[:], n8_2[:])
            norms8 = consts.tile([1, 8], F32, tag="norms8")
            nc.default_dma_engine.dma_start(
                norms8[:],
                nbounce[:].rearrange("z c -> (z c)").rearrange("(o f) -> o f", o=1),
            )
            nmax = consts.tile([1, 8], F32, tag="nmax")
            nidx = consts.tile([1, 8], U32, tag="nidx")
            nc.vector.max(nmax[:], norms8[:])
            nc.vector.max_index(nidx[:], nmax[:], norms8[:])

            if STAGE == 4:
                nc.default_dma_engine.dma_start(dbg32[:], norms8[:])
                nc.default_dma_engine.dma_start(dbgidx[:], nidx[:])
                return

            # ---- extraction: 8 predicated DMAs stored -> out ----
            nc.default_dma_engine.dma_start(out_idx[:], nidx[:])
            PBSZ = NT * PB_STRIDE  # 18432
            rp = nc.alloc_registers("rp")
            rpb = nc.alloc_registers("rpb")
            nc.regs_load(rp, nidx[0:1, 0:1])
            nc.regs_alu(rpb, rp, 3, ALU.bitwise_and)  # pb = rp & 3
            nc.regs_alu(rpb, rpb, PBSZ, ALU.mult)  # pb * PBSZ
            pb_off = nc.snap(rpb, min_val=0, max_val=3 * PBSZ)
            # gather winner pb block in 4 chunks via dynamic-offset engine
            # copies (DVE/ACT alternating), then plain static DMAs out
            GCH = PBSZ // 8  # 2304
            stgs = [
                work_pool.tile([96, GCH], BF16, tag=f"stg{i}", name=f"stg{i}")
                for i in range(4)
            ]
            for g8 in range(8):
                stg = stgs[g8 % 4]
                if PHASE_FORCE >= 0:
                    src = stored[
                        0:96, (PHASE_FORCE & 3) * PBSZ + g8 * GCH :
                    ][:, 0:GCH]
                else:
                    src = stored[0:96, bass.ds(pb_off, PBSZ)][
                        :, g8 * GCH : (g8 + 1) * GCH
                    ]
                nc.vector.tensor_copy(stg[:], src)
                eng = nc.sync if g8 % 2 == 0 else nc.scalar
                eng.dma_start(out[:, g8 * GCH : (g8 + 1) * GCH], stg[:])


_NC_CACHE = {}


def _get_nc():
    if "nc" not in _NC_CACHE:
        nc = bass.Bass()
        build_kernel(nc)
        _split_waits(nc)
        _NC_CACHE["nc"] = nc
    return _NC_CACHE["nc"]


def run(input_to_pool, filt, trace=False):
    import ml_dtypes

    W, P = build_weights(np.asarray(filt))
    nc = _get_nc()
    x = np.ascontiguousarray(np.asarray(input_to_pool, dtype=np.float32))
    B = x.shape[0]
    in_maps = []
    for b in range(B):
        in_maps.append(
            {
                "x": x[b].reshape(C, N, YX),
                "w": W.astype(ml_dtypes.bfloat16),
                "w2": (2.0 * W).astype(ml_dtypes.bfloat16),
                "par": P,
                "parbf": P.astype(ml_dtypes.bfloat16),
                "zeros": np.zeros((32, USZ), dtype=ml_dtypes.bfloat16),
            }
        )
    res = run_bass_kernel_spmd(nc, in_maps, core_ids=list(range(B)), trace=trace)
    if STAGE >= 5:
        outs = np.empty((B, C, NH, NH, NH), dtype=np.float32)
        for b in range(B):
            o2 = np.asarray(res.results[b]["out"], dtype=np.float32).reshape(
                2, 2, NH, NT, NH, NH
            )  # [cl, dz, z', tt, y', x']
            idx = int(np.asarray(res.results[b]["out_idx"]).reshape(8)[0])
            dz = (idx >> 2) & 1
            # -> out[2*tt+cl, z', y', x']
            outs[b, 0::2] = o2[0, dz].transpose(1, 0, 2, 3)
            outs[b, 1::2] = o2[1, dz].transpose(1, 0, 2, 3)
    else:
        outs = None
    return outs, res


def kernel(input_to_pool, filt, permute_indices=None):
    """Full-input entry point: (8,64,48,48,48) f32 -> (8,64,24,24,24) f32."""
    outs, _ = run(input_to_pool, filt, trace=False)
    return outs
